# revision 28
# baseline (speedup 1.0000x reference)
"""CharRNN Trainium2 kernel.

Math (per reference):
    xp = x @ W_ih.T + b_ih + b_hh            # [B,T,H]
    h_t = tanh(xp_t + h_{t-1} @ W_hh.T)      # scan over T
    logits = outs @ W_fc.T + b_fc            # [B,T,V]

Strategy: data-parallel over batch B=128 across 8 cores (B_loc=16/core).
All device compute in a transposed domain: hidden state kept as
h_T[h, b] with H split into 4 chunks of 128 partitions, so the
recurrence matmul is out[h'_chunk, b] += W_tile[k,m].T @ h_T[k, b]
with W_hh tiles as the PE stationary operand (fp16 -> FWL 2x weight
load) and no per-step transposes.  xp is preloaded into PSUM each step
via an identity matmul (start=True), the 16 W-matmuls accumulate on
top, and 4 per-chunk ACT tanh ops write h_T directly into a rolling
SBUF window that doubles as the lhsT source for the fused fc
projection (every 8 steps -> one [128 rows, 96] logits block).

Layouts (device, per core; all host-packed):
  x_t     [T*B, V]   fp32  t-major rows (t*B+b)
  h0t     [128, 4B]  fp16  h0t[p, c*B+b] = h0[b, c*128+p]
  w_ih_p  [97, 512]  fp16  rows 0..95 = W_ih.T, row 96 = b_ih+b_hh
  w_hh_p  [128,2048] fp16  w_hh_p[k, (j*4+c)*128+m] = W_hh[j*128+m, c*128+k]
  w_fc_p  [128, 384] fp16  w_fc_p[p, j*96+v] = W_fc[v, j*128+p]
  b_fc_p  [128, 96]  fp32  broadcast
  logits_t [T*B, V]  fp32  t-major rows (output)
  h_last  [128, 4B]  fp32  same layout as h0t (output)
"""

import numpy as np

import concourse.bass as bass
import concourse.mybir as mybir
import concourse.tile as tile
from concourse import bacc
from concourse.bass_utils import run_bass_kernel_spmd
from concourse.masks import make_identity

F16 = mybir.dt.float16
F32 = mybir.dt.float32

V = 96
H = 512
HC = 4            # H chunks of 128
N_CORES = 8

_CACHE = {}


def build(
    T, B, repeat=1, act_groups=1, xp_add="idmm", hbufs=4, p1copy="dve",
    mode="full",
):
    """Build the Bass program for one core: B local batch, T timesteps.

    repeat>1 re-runs phases 2+3 (identical recompute) for differential
    on-device timing.  act_groups: how many tanh ACT ops per step the 4
    h-chunks are split into (1, 2, or 4).  xp_add: "idmm" preloads xp
    into PSUM via an identity matmul; "dve" accumulates the W matmuls
    first and adds xp with a DVE tensor_tensor afterwards."""
    CB = HC * B                # cols per step slot (4 chunks x B)
    TPB = 128 // B             # timesteps per 128-row block
    NBLK = (T * B) // 128      # number of 128-row blocks
    WIN = 4 * TPB              # rolling window in steps (multiple of TPB)
    assert T % TPB == 0

    nc = bacc.Bacc(None)

    x_t = nc.dram_tensor("x_t", [T * B, V], F32, kind="ExternalInput")
    h0t = nc.dram_tensor("h0t", [128, CB], F16, kind="ExternalInput")
    w_ih = nc.dram_tensor("w_ih_p", [V + 1, H], F16, kind="ExternalInput")
    w_hh = nc.dram_tensor("w_hh_p", [128, 16 * 128], F16, kind="ExternalInput")
    w_fc = nc.dram_tensor("w_fc_p", [128, HC * V], F16, kind="ExternalInput")
    b_fc = nc.dram_tensor("b_fc_p", [128, V], F32, kind="ExternalInput")
    logits = nc.dram_tensor("logits_t", [T * B, V], F32, kind="ExternalOutput")
    h_last = nc.dram_tensor("h_last", [128, CB], F32, kind="ExternalOutput")

    with tile.TileContext(nc) as tc:
        with tc.tile_pool(name="const", bufs=1) as const:
            # ---- constants / persistent buffers ----
            w_hh_sb = const.tile([128, 16 * 128], F16)
            nc.sync.dma_start(out=w_hh_sb, in_=w_hh[:])
            w_ih_sb = const.tile([V + 1, H], F16)
            nc.sync.dma_start(out=w_ih_sb, in_=w_ih[:])
            w_fc_sb = const.tile([128, HC * V], F16)
            nc.sync.dma_start(out=w_fc_sb, in_=w_fc[:])
            b_fc_sb = const.tile([128, V], F32)
            nc.sync.dma_start(out=b_fc_sb, in_=b_fc[:])
            h0_sb = const.tile([128, CB], F16)
            nc.sync.dma_start(out=h0_sb, in_=h0t[:])

            ident = const.tile([128, 128], F32)
            make_identity(nc, ident)
            ident16 = const.tile([128, 128], F16)
            nc.vector.tensor_copy(out=ident16, in_=ident)

            # xp storage, step-major: xp_all[p, t*CB + j*B + b]
            xp_all = const.tile([128, T * CB], F16)
            xp_w = xp_all.rearrange("p (t j b) -> p t j b", j=HC, b=B)

            # rolling h window, chunk-major: outs_win[p, j*(WIN*B) + slot*B + b]
            outs_win = const.tile([128, HC * WIN * B], F16)
            ow = outs_win.rearrange("p (j s b) -> p j s b", j=HC, b=B)

            # x staging: [128, 2 halves, 97]; col 96 is constant 1.0
            x_stage = const.tile([128, 2, V + 1], F32)
            nc.gpsimd.memset(x_stage[:, 0, V : V + 1], 1.0)
            nc.gpsimd.memset(x_stage[:, 1, V : V + 1], 1.0)

            if mode == "null":
                hl0 = const.tile([128, CB], F32)
                nc.vector.tensor_copy(out=hl0, in_=h0_sb)
                nc.sync.dma_start(out=h_last[:], in_=hl0)
                nc.sync.dma_start(out=logits[0:128, :], in_=b_fc_sb)

            # ---- phase 1: xp = (x | 1) @ (W_ih.T | bias), transposed ----
            for _rep1 in range(repeat if mode != "null" else 0):
              with (
                tc.tile_pool(name="p1sb", bufs=3) as p1sb,
                tc.tile_pool(name="p1ps", bufs=2, space="PSUM") as p1ps,
            ):
                for i in range(NBLK):
                    half = i % 2
                    nc.sync.dma_start(
                        out=x_stage[:, half, 0:V],
                        in_=x_t[i * 128 : (i + 1) * 128, :],
                    )
                    xT_ps = p1ps.tile([V + 1, 128], F32, tag="xT_ps")
                    nc.tensor.transpose(xT_ps, x_stage[:, half, :], ident)
                    xT = p1sb.tile([V + 1, 128], F16)
                    nc.scalar.copy(xT, xT_ps)
                    xp_ps = p1ps.tile([128, 512], F32, tag="xp_ps")
                    for j in range(HC):
                        nc.tensor.matmul(
                            xp_ps[:, j * 128 : (j + 1) * 128],
                            lhsT=w_ih_sb[:, j * 128 : (j + 1) * 128],
                            rhs=xT,
                            start=True,
                            stop=True,
                        )
                    if p1copy == "dve":
                        nc.vector.tensor_copy(
                            out=xp_w[:, i * TPB : (i + 1) * TPB, :, :],
                            in_=xp_ps.rearrange(
                                "p (j s b) -> p s j b", j=HC, b=B
                            ),
                        )
                    elif p1copy == "act":
                        nc.scalar.copy(
                            out=xp_w[:, i * TPB : (i + 1) * TPB, :, :],
                            in_=xp_ps.rearrange(
                                "p (j s b) -> p s j b", j=HC, b=B
                            ),
                        )
                    else:  # split: 2 chunks on DVE, 2 on ACT
                        nc.vector.tensor_copy(
                            out=xp_w[:, i * TPB : (i + 1) * TPB, 0:2, :],
                            in_=xp_ps.rearrange(
                                "p (j s b) -> p s j b", j=HC, b=B
                            )[:, :, 0:2, :],
                        )
                        nc.scalar.copy(
                            out=xp_w[:, i * TPB : (i + 1) * TPB, 2:4, :],
                            in_=xp_ps.rearrange(
                                "p (j s b) -> p s j b", j=HC, b=B
                            )[:, :, 2:4, :],
                        )

            if mode == "p1only":
                # keep phase 1 live: consume xp_all into h_last
                hl1 = const.tile([128, CB], F32)
                nc.vector.tensor_copy(
                    out=hl1, in_=xp_all[:, (T - 1) * CB : T * CB]
                )
                nc.sync.dma_start(out=h_last[:], in_=hl1)
                nc.sync.dma_start(out=logits[0:128, :], in_=b_fc_sb)

            # ---- phase 2 + fused phase 3 ----
            if mode == "full":
             with (
                tc.tile_pool(name="hps", bufs=hbufs, space="PSUM") as hps,
                tc.tile_pool(name="fcps", bufs=2, space="PSUM") as fcps,
                tc.tile_pool(name="p3sb", bufs=4) as p3sb,
             ):
              for _rep in range(repeat):
               for t in range(T):
                slot = t % WIN
                pslot = (t - 1) % WIN

                ps = hps.tile([128, CB], F32)
                if xp_add == "idmm":
                    # preload xp_t into PSUM (clears has_written on the bank)
                    nc.tensor.matmul(
                        ps,
                        lhsT=ident16,
                        rhs=xp_all[:, t * CB : (t + 1) * CB],
                        start=True,
                        stop=False,
                        skip_group_check=True,
                    )
                for j in range(HC):
                    for c in range(HC):
                        if t == 0:
                            rhs = h0_sb[:, c * B : (c + 1) * B]
                        else:
                            rhs = ow[:, c, pslot, :]
                        nc.tensor.matmul(
                            ps[:, j * B : (j + 1) * B],
                            lhsT=w_hh_sb[:, (j * 4 + c) * 128 : (j * 4 + c + 1) * 128],
                            rhs=rhs,
                            start=(xp_add != "idmm" and c == 0),
                            stop=(j == HC - 1 and c == HC - 1),
                            skip_group_check=True,
                        )
                if xp_add == "dve":
                    nc.vector.tensor_add(
                        ps, ps, xp_all[:, t * CB : (t + 1) * CB]
                    )
                if act_groups == "esc":
                    bounds = [(0, 1), (1, 2), (2, 4)]
                else:
                    gs = HC // act_groups  # chunks per ACT op
                    bounds = [(g * gs, (g + 1) * gs) for g in range(act_groups)]
                for lo, hi in bounds:
                    nc.scalar.activation(
                        out=ow[:, lo:hi, slot, :],
                        in_=ps[:, lo * B : hi * B],
                        func=mybir.ActivationFunctionType.Tanh,
                    )

                if (t + 1) % TPB == 0:
                    i = t // TPB
                    s0 = (t + 1 - TPB) % WIN
                    fc_ps = fcps.tile([128, V], F32)
                    for j in range(HC):
                        base = j * WIN * B
                        nc.tensor.matmul(
                            fc_ps,
                            lhsT=outs_win[:, base + s0 * B : base + (s0 + TPB) * B],
                            rhs=w_fc_sb[:, j * V : (j + 1) * V],
                            start=(j == 0),
                            stop=(j == HC - 1),
                        )
                    lg = p3sb.tile([128, V], F32)
                    nc.vector.tensor_add(lg, fc_ps, b_fc_sb)
                    nc.sync.dma_start(
                        out=logits[i * 128 : (i + 1) * 128, :], in_=lg
                    )

              hl = p3sb.tile([128, CB], F32, tag="hl")
              nc.vector.tensor_copy(
                  out=hl.rearrange("p (j b) -> p j b", j=HC),
                  in_=ow[:, :, (T - 1) % WIN, :],
              )
              nc.sync.dma_start(out=h_last[:], in_=hl)

    nc.finalize()
    return nc


def _pack_weights(W_ih, W_hh, b_ih, b_hh, W_fc, b_fc):
    w_ih_p = np.concatenate(
        [W_ih.T, (b_ih + b_hh)[None, :]], axis=0
    ).astype(np.float16)                                   # [97, 512]
    # w_hh_p[k, (j*4+c)*128+m] = W_hh[j*128+m, c*128+k]
    Wb = W_hh.reshape(HC, 128, HC, 128)                    # [j, m, c, k]
    w_hh_p = np.ascontiguousarray(
        Wb.transpose(3, 0, 2, 1)                           # [k, j, c, m]
    ).reshape(128, 16 * 128).astype(np.float16)
    w_fc_p = np.ascontiguousarray(
        W_fc.reshape(V, HC, 128).transpose(2, 1, 0)        # [p, j, v]
    ).reshape(128, HC * V).astype(np.float16)
    b_fc_p = np.ascontiguousarray(
        np.broadcast_to(b_fc.astype(np.float32), (128, V))
    )
    return w_ih_p, w_hh_p, w_fc_p, b_fc_p


def kernel(x, hidden, W_ih, W_hh, b_ih, b_hh, W_fc, b_fc, trace=False):
    x = np.asarray(x, np.float32)
    hidden = np.asarray(hidden, np.float32)
    B_tot, T, _ = x.shape
    B = B_tot // N_CORES
    CB = HC * B

    key = (T, B)
    if key not in _CACHE:
        _CACHE[key] = build(T, B)
    nc = _CACHE[key]

    w_ih_p, w_hh_p, w_fc_p, b_fc_p = _pack_weights(
        np.asarray(W_ih, np.float32), np.asarray(W_hh, np.float32),
        np.asarray(b_ih, np.float32), np.asarray(b_hh, np.float32),
        np.asarray(W_fc, np.float32), np.asarray(b_fc, np.float32),
    )

    in_maps = []
    for c in range(N_CORES):
        xs = x[c * B : (c + 1) * B]                        # [B, T, V]
        x_t = np.ascontiguousarray(xs.transpose(1, 0, 2)).reshape(T * B, V)
        h0 = hidden[0, c * B : (c + 1) * B]                # [B, H]
        h0t = np.ascontiguousarray(
            h0.reshape(B, HC, 128).transpose(2, 1, 0)      # [p, j, b]
        ).reshape(128, CB).astype(np.float16)
        in_maps.append({
            "x_t": x_t,
            "h0t": h0t,
            "w_ih_p": w_ih_p,
            "w_hh_p": w_hh_p,
            "w_fc_p": w_fc_p,
            "b_fc_p": b_fc_p,
        })

    res = run_bass_kernel_spmd(
        nc, in_maps, core_ids=list(range(N_CORES)), trace=trace
    )

    logits = np.empty((B_tot, T, V), np.float32)
    h_out = np.empty((1, B_tot, H), np.float32)
    for c in range(N_CORES):
        r = res.results[c]
        lt = r["logits_t"].reshape(T, B, V)
        logits[c * B : (c + 1) * B] = lt.transpose(1, 0, 2)
        hl = r["h_last"].reshape(128, HC, B)               # [p, j, b]
        h_out[0, c * B : (c + 1) * B] = hl.transpose(2, 1, 0).reshape(B, H)

    kernel.last_result = res
    return logits, h_out
